# revision 16
# baseline (speedup 1.0000x reference)
"""Trainium2 Bass kernel for nn_GAT_1580547974673 (2-layer GAT + pair scoring).

Self-contained: hardcodes all shapes/sharding. Strategy: row-shard the NxN
attention over 8 cores (384 rows each, all 8 heads), pair scoring sharded
over P. Three AllGathers stitch the distributed pieces (h, layer-2 h, x_out).

Math restructuring (validated vs reference in fp64/fp32 numpy to ~3e-6):
  - f1 = x @ (W @ a1), f2 = x @ (W @ a2)         (weight folding)
  - att_unnorm = exp(lrelu(f1_i + f2_j + M_ij)), M = 0 / -1e9 (mask pre-fold;
    exp of masked entries underflows to exactly 0)
  - no max-subtraction (|z| <= ~25, exp stays in fp32 range)
  - rowsum via ones-augmented h in the att @ [h|1] matmul; divide after
  - elu(t) = relu(t) + min(exp(t), 1) - 1        (single Exp, fused combine)
Layout: attention computed transposed (j on partitions, i on free dim) so the
contraction dim of att @ h lands on partitions; per-partition scalars are f2,
free-dim broadcast of f1 built once per head via gpsimd partition_broadcast.
"""
import numpy as np
from contextlib import ExitStack

import concourse.bass as bass
import concourse.bacc as bacc
import concourse.mybir as mybir
import concourse.tile as tile
import concourse.dve_ops as dve_ops
from concourse.dve_ops import DveOp, OPS
from concourse.dve_spec import Spec, Src0, Src1, C0, C1, One, maxx, minn, relu, lower
from concourse.dve_uop import DveOpSpec
from concourse.bass_utils import run_bass_kernel_spmd
from concourse.masks import make_identity

F32 = mybir.dt.float32
F32R = mybir.dt.float32r
AF = mybir.ActivationFunctionType

# problem shapes (hardcoded per spec)
N, FIN, FH, H, NPAIR = 3072, 512, 64, 8, 2048
NC = 8
IB = N // NC            # 384 rows per core
PB = NPAIR // NC        # 256 pairs per core
NJ = N // 128           # 24 j-blocks
KB = FIN // 128         # 4 k-blocks of the feature dim
SUB = IB // 128         # 3 sub-blocks of the core's row slice
CH = 4                  # j-blocks per exp chunk
NCH = NJ // CH
MASKVAL = -1.0e9
ALPHA = 0.2

# If DMA-produced float32r operands pass the BIR verifier, we skip explicit
# rounding copies. Set by probing (see _probe note); default False = safe.
F32R_DMA_OK = True
DEBUG = False


def _register_ops():
    """Register the two custom DVE ops (idempotent)."""
    defs = []
    if "GAT_MASK_LRELU" not in dve_ops._SUB_OPCODE_FOR_NAME:
        s = (Src0 + Src1) + C0
        defs.append(DveOp(
            "GAT_MASK_LRELU",
            Spec(body=maxx(s, s * C1),
                 reference=lambda in0, in1, s0, s1, imm2: np.maximum(
                     (in0 + in1) + s0, ((in0 + in1) + s0) * s1)),
            subdim=False, uops_sha={}))
    if "GAT_ELU_COMBINE" not in dve_ops._SUB_OPCODE_FOR_NAME:
        # out = relu(t) + min(E, 1) - 1  with t=Src0, E=Src1(=exp(t))
        defs.append(DveOp(
            "GAT_ELU_COMBINE",
            Spec(body=relu(Src0) + minn(Src1, One) - One,
                 reference=lambda in0, in1, s0, s1, imm2:
                     np.maximum(in0, 0) + np.minimum(in1, 1.0) - 1.0),
            subdim=False, uops_sha={}))
    for op in defs:
        for ver in ("v3", "v4"):
            tmp = DveOpSpec(name=op.name, opcode=0,
                            uops=lower(op.spec, ver=ver), rd1_en=True)
            op.uops_sha[ver] = tmp.sha(ver)
        dve_ops.OPS.append(op)
        dve_ops.CUSTOM_DVE_SPECS[op.name] = op.spec
        dve_ops._SUB_OPCODE_FOR_NAME[op.name] = (
            dve_ops._CUSTOM_DVE_ROW_BASE + len(dve_ops.OPS) - 1)
    ops = {op.name: op for op in dve_ops.OPS}
    return ops["GAT_MASK_LRELU"], ops["GAT_ELU_COMBINE"]


def build(nc):
    op_mask_lrelu, op_elu = _register_ops()
    HDT = F32R if F32R_DMA_OK else F32  # dtype for gathered matmul operands

    # ---- I/O ----
    xTs_in = nc.dram_tensor("xTs_in", [FIN, IB], F32, kind="ExternalInput")
    maskT_in = nc.dram_tensor("maskT_in", [N, IB], F32, kind="ExternalInput")
    Wall_in = nc.dram_tensor("Wall_in", [FIN, FIN], F32, kind="ExternalInput")
    W12_in = nc.dram_tensor("W12_in", [FIN, 2 * H], F32, kind="ExternalInput")
    Wo_in = nc.dram_tensor("Wo_in", [FIN, FH + 2], F32, kind="ExternalInput")
    wgt_in = nc.dram_tensor("wgt_in", [FH, FH], F32, kind="ExternalInput")
    p1T_in = nc.dram_tensor("p1T_in", [N, PB], F32R, kind="ExternalInput")
    p2T_in = nc.dram_tensor("p2T_in", [N, PB], F32R, kind="ExternalInput")
    scores_out = nc.dram_tensor("scores_out", [1, PB], F32, kind="ExternalOutput")
    dbg = {}
    if DEBUG:
        for nm, shp in [("d_ft", [2 * H, IB]), ("d_f2", [128, H]),
                        ("d_haug", [128, H * (FH + 1)]), ("d_xct", [128, IB]),
                        ("d_h2", [128, FH + 2]), ("d_xo", [128, FH]),
                        ("d_e1", [FH, PB]), ("d_hp", [FH + 1, IB]),
                        ("d_zt", [128, IB]), ("d_et", [128, IB])]:
            dbg[nm] = nc.dram_tensor(nm, shp, F32, kind="ExternalOutput")

    groups = [list(range(NC))]

    with tile.TileContext(nc) as tc, ExitStack() as ctx:
        tiny = ctx.enter_context(tc.tile_pool(name="tiny", bufs=1))
        xcp = ctx.enter_context(tc.tile_pool(name="xcp", bufs=1))
        h2pool = ctx.enter_context(tc.tile_pool(name="h2pool", bufs=1))
        xopool = ctx.enter_context(tc.tile_pool(name="xopool", bufs=1))
        epool = ctx.enter_context(tc.tile_pool(name="epool", bufs=1))
        npool = ctx.enter_context(tc.tile_pool(name="npool", bufs=1))
        dram = ctx.enter_context(tc.tile_pool(name="dram", bufs=1, space="DRAM"))
        ps_small = ctx.enter_context(tc.tile_pool(name="ps_small", bufs=2, space="PSUM"))
        ps_h_pool = ctx.enter_context(tc.tile_pool(name="ps_h", bufs=2, space="PSUM"))
        ps_hp_pool = ctx.enter_context(tc.tile_pool(name="ps_hp", bufs=4, space="PSUM"))
        ctx_att1 = ExitStack()
        maskp = ctx_att1.enter_context(tc.tile_pool(name="maskp", bufs=1))
        ztp = ctx_att1.enter_context(tc.tile_pool(name="ztp", bufs=2))
        ep = ctx_att1.enter_context(tc.tile_pool(name="ep", bufs=2))
        ctx_prep = ExitStack()
        cst = ctx_prep.enter_context(tc.tile_pool(name="cst", bufs=1))
        fpool = ctx_prep.enter_context(tc.tile_pool(name="fpool", bufs=1))
        hpool = ctx_prep.enter_context(tc.tile_pool(name="hpool", bufs=1))

        # ---- constant loads ----
        xTs = []
        Wall = []
        W12 = []
        Wo = []
        for kb in range(KB):
            t1 = cst.tile([128, IB], F32, name=f"xTs{kb}")
            nc.sync.dma_start(t1[:], xTs_in[kb * 128:(kb + 1) * 128, :])
            xTs.append(t1)
            t2 = cst.tile([128, FIN], F32, name=f"Wall{kb}")
            nc.sync.dma_start(t2[:], Wall_in[kb * 128:(kb + 1) * 128, :])
            Wall.append(t2)
            t3 = cst.tile([128, 2 * H], F32, name=f"W12_{kb}")
            nc.sync.dma_start(t3[:], W12_in[kb * 128:(kb + 1) * 128, :])
            W12.append(t3)
            t4 = tiny.tile([128, FH + 2], F32, name=f"Wo{kb}")
            nc.sync.dma_start(t4[:], Wo_in[kb * 128:(kb + 1) * 128, :])
            Wo.append(t4)
        wgt = tiny.tile([FH, FH], F32)
        nc.sync.dma_start(wgt[:], wgt_in[:])
        ones8 = tiny.tile([128, H], F32)
        nc.gpsimd.memset(ones8[:], 1.0)
        ones64 = tiny.tile([FH, 1], F32)
        nc.gpsimd.memset(ones64[:], 1.0)
        ident = tiny.tile([128, 128], F32)
        make_identity(nc, ident[:])

        # mask tiles (stay resident through both attention layers)
        maskT = []
        for jb in range(NJ):
            m = maskp.tile([128, IB], F32, name=f"maskT{jb}")
            nc.sync.dma_start(m[:], maskT_in[jb * 128:(jb + 1) * 128, :])
            maskT.append(m)

        # ---- stage B: F1 (local rows, free-dim layout) + F2 (local, gathered) ----
        ps_ft = ps_small.tile([2 * H, IB], F32, tag="pss")
        for kb in range(KB):
            nc.tensor.matmul(ps_ft[:], W12[kb][:], xTs[kb][:],
                             start=(kb == 0), stop=(kb == KB - 1))
        FTsb = fpool.tile([2 * H, IB], F32)
        nc.scalar.copy(FTsb[:], ps_ft[:])
        ft_d = dram.tile([2 * H, IB], F32)
        nc.sync.dma_start(ft_d[:], FTsb[:])

        F2loc_sb = []
        for s in range(SUB):
            ps_f2 = ps_small.tile([128, H], F32, tag="pss")
            for kb in range(KB):
                nc.tensor.matmul(ps_f2[:], xTs[kb][:, s * 128:(s + 1) * 128],
                                 W12[kb][:, H:2 * H],
                                 start=(kb == 0), stop=(kb == KB - 1))
            t = fpool.tile([128, H], F32, name=f"F2loc{s}")
            nc.scalar.copy(t[:], ps_f2[:])
            F2loc_sb.append(t)

        # ---- stage C: local h (fp32, exact) -> haug layout -> gather ----
        hloc_sb = []
        for s in range(SUB):
            ps_h = ps_h_pool.tile([128, FIN], F32, tag="ph")
            for kb in range(KB):
                nc.tensor.matmul(ps_h[:], xTs[kb][:, s * 128:(s + 1) * 128],
                                 Wall[kb][:],
                                 start=(kb == 0), stop=(kb == KB - 1))
            hsb = fpool.tile([128, H * (FH + 1)], HDT, name=f"hloc{s}")
            hsb3 = hsb[:].rearrange("p (h f) -> p h f", h=H)
            nc.scalar.copy(hsb3[:, :, 0:FH],
                           ps_h[:].rearrange("p (h f) -> p h f", h=H))
            nc.scalar.copy(hsb3[:, :, FH], ones8[:])
            hloc_sb.append(hsb)

        # gathers: hloc (big) and F2loc (small, fp32-exact)
        hloc_d = dram.tile([IB, H * (FH + 1)], HDT)
        hg_d = dram.tile([N, H * (FH + 1)], HDT, addr_space="Shared")
        for s in range(SUB):
            nc.sync.dma_start(hloc_d[s * 128:(s + 1) * 128, :], hloc_sb[s][:])
        nc.gpsimd.collective_compute(
            "AllGather", mybir.AluOpType.bypass, replica_groups=groups,
            ins=[hloc_d[:].opt()], outs=[hg_d[:].opt()])

        f2loc_d = dram.tile([IB, H], F32)
        f2g_d = dram.tile([N, H], F32, addr_space="Shared")
        for s in range(SUB):
            nc.sync.dma_start(f2loc_d[s * 128:(s + 1) * 128, :], F2loc_sb[s][:])
        nc.gpsimd.collective_compute(
            "AllGather", mybir.AluOpType.bypass, replica_groups=groups,
            ins=[f2loc_d[:].opt()], outs=[f2g_d[:].opt()])

        # DMA gathered tensors back per j-block
        haug = []
        F2sb = []
        for jb in range(NJ):
            ht = hpool.tile([128, H * (FH + 1)], HDT, name=f"haug{jb}")
            nc.sync.dma_start(ht[:], hg_d[jb * 128:(jb + 1) * 128, :])
            f2t = hpool.tile([128, H], F32, name=f"F2sb{jb}")
            nc.sync.dma_start(f2t[:], f2g_d[jb * 128:(jb + 1) * 128, :])
            haug.append(ht)
            F2sb.append(f2t)
        if not F32R_DMA_OK:
            haug_r = []
            for jb in range(NJ):
                hr = hpool.tile([128, H * (FH + 1)], F32R, name=f"haugr{jb}")
                nc.vector.tensor_copy(hr[:], haug[jb][:])
                haug_r.append(hr)
        else:
            haug_r = haug

        # f1 broadcast tiles per head (row bounced to partition 0 via DRAM)
        f1b = []
        for h in range(H):
            row = fpool.tile([1, IB], F32, name=f"f1row{h}")
            nc.sync.dma_start(row[:], ft_d[h:h + 1, :])
            t = fpool.tile([128, IB], F32, name=f"f1b{h}")
            nc.gpsimd.partition_broadcast(t[:], row[:])
            f1b.append(t)

        # ---- stage D: layer-1 attention, per head ----
        xcT = [xcp.tile([128, IB], F32, name=f"xcT{kb}") for kb in range(KB)]

        def attention(head, haug_col0, f2col_of, f1b_t, out_sb, out_p0):
            """One attention unit: out_sb[out_p0:out_p0+FH, :] = elu(att @ h)."""
            ps_hp = ps_hp_pool.tile([FH + 1, IB], F32, tag="hp",
                                    name=f"ps_hp{head}")
            for c in range(NCH):
                zt = ztp.tile([128, CH, IB], F32, tag="zt", name=f"zt{head}_{c}")
                for g in range(CH):
                    jb = c * CH + g
                    nc.vector._custom_dve(
                        op_mask_lrelu, out=zt[:, g, :], in0=f1b_t[:],
                        in1=maskT[jb][:], s0=f2col_of(jb), s1=ALPHA)
                et = ep.tile([128, CH, IB], F32R, tag="et", name=f"et{head}_{c}")
                nc.scalar.activation(et[:], zt[:], AF.Exp)
                if DEBUG and head == 0 and c == 0:
                    nc.sync.dma_start(dbg["d_zt"][:], zt[:, 0, :])
                    nc.sync.dma_start(dbg["d_et"][:], et[:, 0, :].bitcast(F32))
                for g in range(CH):
                    jb = c * CH + g
                    nc.tensor.matmul(
                        ps_hp[:], haug_r[jb][:, haug_col0:haug_col0 + FH + 1],
                        et[:, g, :], start=(jb == 0), stop=(jb == NJ - 1))
            # normalize + elu
            if DEBUG and head == 0:
                hp_sb = npool.tile([FH + 1, IB], F32, tag="dbg_hp")
                nc.scalar.copy(hp_sb[:], ps_hp[:])
                nc.sync.dma_start(dbg["d_hp"][:], hp_sb[:])
            # rowsum lives at PSUM partition FH; copy within-partition to
            # SBUF, then DMA (the only partition-moving engine) to partition 0
            rs64 = npool.tile([128, IB], F32, tag="rs64", name=f"rs64_{head}")
            nc.scalar.copy(rs64[FH:FH + 1, :], ps_hp[FH:FH + 1, :])
            rsum = npool.tile([1, IB], F32, tag="rsum", name=f"rsum{head}")
            nc.sync.dma_start(rsum[:], rs64[FH:FH + 1, :])
            rrow = npool.tile([1, IB], F32, tag="rrow", name=f"rrow{head}")
            nc.vector.reciprocal(rrow[:], rsum[:])
            rb = npool.tile([FH, IB], F32, tag="rb", name=f"rb{head}")
            nc.gpsimd.partition_broadcast(rb[:], rrow[:])
            t_n = npool.tile([FH, IB], F32, tag="tn", name=f"tn{head}")
            nc.vector.tensor_mul(t_n[:], ps_hp[0:FH, :], rb[:])
            e_n = npool.tile([FH, IB], F32, tag="en", name=f"en{head}")
            nc.scalar.activation(e_n[:], t_n[:], AF.Exp)
            eluo = npool.tile([FH, IB], F32, tag="eluo", name=f"eluo{head}")
            nc.vector._custom_dve(op_elu, out=eluo[:], in0=t_n[:], in1=e_n[:])
            nc.sync.dma_start(out_sb[out_p0:out_p0 + FH, :], eluo[:])

        for head in range(H):
            attention(head, head * (FH + 1),
                      lambda jb, h=head: F2sb[jb][:, h:h + 1],
                      f1b[head], xcT[head // 2][:], (head % 2) * FH)

        ctx_prep.close()

        if DEBUG:
            nc.sync.dma_start(dbg["d_ft"][:], FTsb[:])
            nc.sync.dma_start(dbg["d_f2"][:], F2sb[0][:])
            nc.sync.dma_start(dbg["d_haug"][:], haug_r[0][:].bitcast(F32))
            nc.sync.dma_start(dbg["d_xct"][:], xcT[0][:])

        # ---- stage E: layer-2 h (local rows) -> gather ----
        h2loc_sb = []
        for s in range(SUB):
            ps_h2 = ps_small.tile([128, FH + 1], F32, tag="pss")
            for kb in range(KB):
                nc.tensor.matmul(ps_h2[:], xcT[kb][:, s * 128:(s + 1) * 128],
                                 Wo[kb][:, 0:FH + 1],
                                 start=(kb == 0), stop=(kb == KB - 1))
            t = h2pool.tile([128, FH + 2], F32, name=f"h2loc{s}")
            nc.scalar.copy(t[:, 0:FH], ps_h2[:, 0:FH])
            nc.scalar.copy(t[:, FH:FH + 1], ones8[:, 0:1])
            nc.scalar.copy(t[:, FH + 1:FH + 2], ps_h2[:, FH:FH + 1])
            h2loc_sb.append(t)
        # f1_2 as a free-dim row
        ps_f12 = ps_small.tile([1, IB], F32, tag="pss")
        for kb in range(KB):
            nc.tensor.matmul(ps_f12[:], Wo[kb][:, FH + 1:FH + 2], xcT[kb][:],
                             start=(kb == 0), stop=(kb == KB - 1))
        f12row = h2pool.tile([1, IB], F32)
        nc.scalar.copy(f12row[:], ps_f12[:])
        f12b = h2pool.tile([128, IB], F32)
        nc.gpsimd.partition_broadcast(f12b[:], f12row[:])

        h2loc_d = dram.tile([IB, FH + 2], F32)
        h2g_d = dram.tile([N, FH + 2], F32, addr_space="Shared")
        for s in range(SUB):
            nc.sync.dma_start(h2loc_d[s * 128:(s + 1) * 128, :], h2loc_sb[s][:])
        nc.gpsimd.collective_compute(
            "AllGather", mybir.AluOpType.bypass, replica_groups=groups,
            ins=[h2loc_d[:].opt()], outs=[h2g_d[:].opt()])

        h2sb = []
        h2r = []
        for jb in range(NJ):
            t = h2pool.tile([128, FH + 2], F32, name=f"h2sb{jb}")
            nc.sync.dma_start(t[:], h2g_d[jb * 128:(jb + 1) * 128, :])
            h2sb.append(t)
            r = h2pool.tile([128, FH + 1], F32R, name=f"h2r{jb}")
            nc.vector.tensor_copy(r[:], t[:, 0:FH + 1])
            h2r.append(r)

        # ---- stage F: layer-2 attention (single head) ----
        xoT = h2pool.tile([FH, IB], F32)

        ps_hp2 = ps_hp_pool.tile([FH + 1, IB], F32, tag="hp", name="ps_hp2")
        for c in range(NCH):
            zt = ztp.tile([128, CH, IB], F32, tag="zt", name=f"zt2_{c}")
            for g in range(CH):
                jb = c * CH + g
                nc.vector._custom_dve(
                    op_mask_lrelu, out=zt[:, g, :], in0=f12b[:],
                    in1=maskT[jb][:], s0=h2sb[jb][:, FH + 1:FH + 2], s1=ALPHA)
            et = ep.tile([128, CH, IB], F32R, tag="et", name=f"et2_{c}")
            nc.scalar.activation(et[:], zt[:], AF.Exp)
            for g in range(CH):
                jb = c * CH + g
                nc.tensor.matmul(ps_hp2[:], h2r[jb][:], et[:, g, :],
                                 start=(jb == 0), stop=(jb == NJ - 1))
        rs64b = npool.tile([128, IB], F32, tag="rs64", name="rs64b")
        nc.scalar.copy(rs64b[FH:FH + 1, :], ps_hp2[FH:FH + 1, :])
        rsum2 = npool.tile([1, IB], F32, tag="rsum", name="rsum2")
        nc.sync.dma_start(rsum2[:], rs64b[FH:FH + 1, :])
        rrow2 = npool.tile([1, IB], F32, tag="rrow", name="rrow2")
        nc.vector.reciprocal(rrow2[:], rsum2[:])
        rb2 = npool.tile([FH, IB], F32, tag="rb", name="rb2")
        nc.gpsimd.partition_broadcast(rb2[:], rrow2[:])
        t_n2 = npool.tile([FH, IB], F32, tag="tn", name="tn2")
        nc.vector.tensor_mul(t_n2[:], ps_hp2[0:FH, :], rb2[:])
        e_n2 = npool.tile([FH, IB], F32, tag="en", name="en2")
        nc.scalar.activation(e_n2[:], t_n2[:], AF.Exp)
        nc.vector._custom_dve(op_elu, out=xoT[:], in0=t_n2[:], in1=e_n2[:])

        if DEBUG:
            nc.sync.dma_start(dbg["d_h2"][:], h2sb[0][:])
        ctx_att1.close()

        # ---- stage G: x_out natural layout + gather ----
        xoloc_d = dram.tile([IB, FH], F32R)
        for s in range(SUB):
            ps_tr = ps_small.tile([128, FH], F32, tag="pss")
            nc.tensor.transpose(ps_tr[:], xoT[:, s * 128:(s + 1) * 128], ident[0:FH, 0:FH])
            t = xopool.tile([128, FH], F32R, name=f"xol{s}")
            nc.scalar.copy(t[:], ps_tr[:])
            nc.sync.dma_start(xoloc_d[s * 128:(s + 1) * 128, :], t[:])
        xog_d = dram.tile([N, FH], F32R, addr_space="Shared")
        nc.gpsimd.collective_compute(
            "AllGather", mybir.AluOpType.bypass, replica_groups=groups,
            ins=[xoloc_d[:].opt()], outs=[xog_d[:].opt()])

        xor_ = []
        for jb in range(NJ):
            r = xopool.tile([128, FH], F32R, name=f"xor{jb}")
            nc.sync.dma_start(r[:], xog_d[jb * 128:(jb + 1) * 128, :])
            xor_.append(r)

        # ---- stage H: pair embeddings + scores ----
        if DEBUG:
            nc.sync.dma_start(dbg["d_xo"][:], xor_[0][:].bitcast(F32))
        ppool = ctx.enter_context(tc.tile_pool(name="ppool", bufs=1))
        p1sb = []
        p2sb = []
        for jb in range(NJ):
            t1 = ppool.tile([128, PB], F32R if F32R_DMA_OK else F32,
                            name=f"p1_{jb}")
            nc.sync.dma_start(t1[:], p1T_in[jb * 128:(jb + 1) * 128, :])
            t2 = ppool.tile([128, PB], F32R if F32R_DMA_OK else F32,
                            name=f"p2_{jb}")
            nc.sync.dma_start(t2[:], p2T_in[jb * 128:(jb + 1) * 128, :])
            p1sb.append(t1)
            p2sb.append(t2)
        if not F32R_DMA_OK:
            p1r, p2r = [], []
            for jb in range(NJ):
                r1 = ppool.tile([128, PB], F32R, name=f"p1r{jb}")
                nc.vector.tensor_copy(r1[:], p1sb[jb][:])
                r2 = ppool.tile([128, PB], F32R, name=f"p2r{jb}")
                nc.vector.tensor_copy(r2[:], p2sb[jb][:])
                p1r.append(r1)
                p2r.append(r2)
        else:
            p1r, p2r = p1sb, p2sb

        ps_e1 = ps_small.tile([FH, PB], F32, tag="pss", name="ps_e1")
        for jb in range(NJ):
            nc.tensor.matmul(ps_e1[:], xor_[jb][:], p1r[jb][:],
                             start=(jb == 0), stop=(jb == NJ - 1))
        e1sb = epool.tile([FH, PB], F32)
        nc.scalar.copy(e1sb[:], ps_e1[:])
        ps_e2 = ps_small.tile([FH, PB], F32, tag="pss", name="ps_e2")
        for jb in range(NJ):
            nc.tensor.matmul(ps_e2[:], xor_[jb][:], p2r[jb][:],
                             start=(jb == 0), stop=(jb == NJ - 1))
        e2sb = epool.tile([FH, PB], F32)
        nc.scalar.copy(e2sb[:], ps_e2[:])

        ps_g = ps_small.tile([FH, PB], F32, tag="pss", name="ps_g")
        nc.tensor.matmul(ps_g[:], wgt[:], e1sb[:], start=True, stop=True)
        prod = epool.tile([FH, PB], F32)
        nc.vector.tensor_mul(prod[:], ps_g[:], e2sb[:])
        ps_s = ps_small.tile([1, PB], F32, tag="pss", name="ps_s")
        nc.tensor.matmul(ps_s[:], ones64[:], prod[:], start=True, stop=True)
        if DEBUG:
            nc.sync.dma_start(dbg["d_e1"][:], e1sb[:])
        srow = epool.tile([1, PB], F32)
        nc.scalar.copy(srow[:], ps_s[:])
        nc.sync.dma_start(scores_out[:], srow[:])

    return nc


_CACHE = {}


def _get_nc():
    if "nc" not in _CACHE:
        nc = bacc.Bacc(None, target_bir_lowering=False, debug=False, num_devices=NC)
        build(nc)
        nc.compile()
        _CACHE["nc"] = nc
    return _CACHE["nc"]


def prep_inputs(x, adj, pair1_map, pair2_map, Wh, a1h, a2h, W_out, a1_out,
                a2_out, weight):
    x = np.ascontiguousarray(np.asarray(x, np.float32))
    adj = np.asarray(adj)
    maskT = np.where(adj > 0, np.float32(0.0), np.float32(MASKVAL)).T  # [j, i]
    maskT = np.ascontiguousarray(maskT)
    xT = np.ascontiguousarray(x.T)                                     # [FIN, N]
    Wall = np.ascontiguousarray(
        np.transpose(np.asarray(Wh, np.float64), (1, 0, 2)).reshape(FIN, H * FH)
    ).astype(np.float32)
    w1 = np.einsum("hkf,hf->kh", np.asarray(Wh, np.float64), np.asarray(a1h, np.float64))
    w2 = np.einsum("hkf,hf->kh", np.asarray(Wh, np.float64), np.asarray(a2h, np.float64))
    W12 = np.concatenate([w1, w2], axis=1).astype(np.float32)          # [FIN, 16]
    w1o = np.asarray(W_out, np.float64) @ np.asarray(a1_out, np.float64)
    w2o = np.asarray(W_out, np.float64) @ np.asarray(a2_out, np.float64)
    Wo = np.concatenate([np.asarray(W_out, np.float64), w2o[:, None],
                         w1o[:, None]], axis=1).astype(np.float32)     # [FIN, 66]
    p1T = np.ascontiguousarray(np.asarray(pair1_map, np.float32).T)    # [N, NPAIR]
    p2T = np.ascontiguousarray(np.asarray(pair2_map, np.float32).T)
    wgt = np.ascontiguousarray(np.asarray(weight, np.float32))

    in_maps = []
    for c in range(NC):
        i0, i1 = c * IB, (c + 1) * IB
        p0, p1 = c * PB, (c + 1) * PB
        in_maps.append({
            "xTs_in": np.ascontiguousarray(xT[:, i0:i1]),
            "maskT_in": np.ascontiguousarray(maskT[:, i0:i1]),
            "Wall_in": Wall,
            "W12_in": W12,
            "Wo_in": Wo,
            "wgt_in": wgt,
            "p1T_in": np.ascontiguousarray(p1T[:, p0:p1]),
            "p2T_in": np.ascontiguousarray(p2T[:, p0:p1]),
        })
    return in_maps


def run(inputs, trace=False, **kw):
    nc = _get_nc()
    in_maps = prep_inputs(**inputs)
    res = run_bass_kernel_spmd(nc, in_maps, list(range(NC)), trace=trace, **kw)
    scores = np.concatenate(
        [res.results[c]["scores_out"].reshape(-1) for c in range(NC)])
    return scores.astype(np.float32), res


def kernel(**inputs):
    return run(inputs)[0]


def bench(inputs, iters=20, warmup=3):
    """Wall-clock benchmark with device-resident inputs: returns per-iter ns.

    Mirrors bass2jax.run_bass_via_pjrt's multi-core path but keeps inputs on
    device and loops executions, so the measured time is kernel execution +
    dispatch, not host<->device transfer.
    """
    import time
    import jax
    import jax.numpy as jnp
    from jax.sharding import Mesh, PartitionSpec, NamedSharding
    from jax.experimental.shard_map import shard_map
    from concourse import bass2jax
    import concourse.mybir as _mb

    nc = _get_nc()
    in_maps = prep_inputs(**inputs)
    bass2jax.install_neuronx_cc_hook()

    partition_name = nc.partition_id_tensor.name if nc.partition_id_tensor else None
    in_names, out_names, out_avals, zero_outs = [], [], [], []
    for alloc in nc.m.functions[0].allocations:
        if not isinstance(alloc, _mb.MemoryLocationSet):
            continue
        name = alloc.memorylocations[0].name
        if alloc.kind == "ExternalInput":
            if name != partition_name:
                in_names.append(name)
        elif alloc.kind == "ExternalOutput":
            shape = list(alloc.tensor_shape)
            npdt = _mb.dt.np(alloc.dtype)
            out_names.append(name)
            out_avals.append(jax.core.ShapedArray(shape, npdt))
            zero_outs.append(np.zeros(shape, npdt))
    n_params = len(in_names)
    n_outs = len(out_names)
    all_in_names = list(in_names) + list(out_names)
    if partition_name is not None:
        all_in_names.append(partition_name)

    def _body(*args):
        operands = list(args)
        if partition_name is not None:
            operands.append(bass2jax.partition_id_tensor())
        outs = bass2jax._bass_exec_p.bind(
            *operands,
            out_avals=tuple(out_avals),
            in_names=tuple(all_in_names),
            out_names=tuple(out_names),
            lowering_input_output_aliases=(),
            sim_require_finite=True,
            sim_require_nnan=True,
            nc=nc,
        )
        return tuple(outs)

    devices = jax.devices()[:NC]
    mesh = Mesh(np.asarray(devices), ("core",))
    in_specs = (PartitionSpec("core"),) * (n_params + n_outs)
    out_specs = (PartitionSpec("core"),) * n_outs
    fn = jax.jit(shard_map(_body, mesh=mesh, in_specs=in_specs,
                           out_specs=out_specs, check_rep=False),
                 keep_unused=True)
    concat_in = [
        np.concatenate([np.asarray(in_maps[c][nm]) for c in range(NC)], axis=0)
        for nm in in_names
    ]
    concat_zeros = [np.zeros((NC * z.shape[0], *z.shape[1:]), z.dtype)
                    for z in zero_outs]
    sh = NamedSharding(mesh, PartitionSpec("core"))
    dev_in = [jax.device_put(a, sh) for a in concat_in]
    dev_zero = [jax.device_put(a, sh) for a in concat_zeros]

    for _ in range(warmup):
        out = fn(*dev_in, *dev_zero)
        jax.block_until_ready(out)
    # pipelined timing
    t0 = time.perf_counter()
    outs = [fn(*dev_in, *dev_zero) for _ in range(iters)]
    jax.block_until_ready(outs)
    t1 = time.perf_counter()
    per_iter_pipelined = (t1 - t0) / iters
    # serial timing (per-call latency)
    times = []
    for _ in range(iters):
        t0 = time.perf_counter()
        jax.block_until_ready(fn(*dev_in, *dev_zero))
        times.append(time.perf_counter() - t0)
    return dict(pipelined_ns=per_iter_pipelined * 1e9,
                serial_min_ns=min(times) * 1e9,
                serial_med_ns=sorted(times)[len(times) // 2] * 1e9)


if __name__ == "__main__":
    # quick self-drive with random inputs of the right shapes (no reference)
    rng = np.random.default_rng(0)
    ins = dict(
        x=rng.standard_normal((N, FIN), dtype=np.float32),
        adj=(rng.random((N, N)) < 0.5).astype(np.int32),
        pair1_map=rng.standard_normal((NPAIR, N), dtype=np.float32),
        pair2_map=rng.standard_normal((NPAIR, N), dtype=np.float32),
        Wh=rng.standard_normal((H, FIN, FH), dtype=np.float32) * 0.1,
        a1h=rng.standard_normal((H, FH), dtype=np.float32) * 0.3,
        a2h=rng.standard_normal((H, FH), dtype=np.float32) * 0.3,
        W_out=rng.standard_normal((FIN, FH), dtype=np.float32) * 0.1,
        a1_out=rng.standard_normal((FH,), dtype=np.float32) * 0.3,
        a2_out=rng.standard_normal((FH,), dtype=np.float32) * 0.3,
        weight=rng.standard_normal((FH, FH), dtype=np.float32) * 0.1,
    )
    out = kernel(**ins)
    print("scores:", out.shape, out[:8])
